# revision 29
# baseline (speedup 1.0000x reference)
"""Trainium2 Bass kernel for masked pairwise-sigmoid GNN message passing.

Reference computation (per graph g with nodes i,j in [0,nv)):
    c = z @ Wc.T + bc ; y = z @ Wy.T + by          # [G, nv, H]
    s[g,i,j,:] = sigmoid(c[g,i,:] + y[g,j,:] + (m_i + m_j)*L - 2L)
    out[g,i,:] = sum_j s[g,i,j,:] / sum_j m[g,j]

Exact identity: with m in {0,1}, any pair with m_i==0 or m_j==0 has mask
term <= -1e10, so sigmoid underflows to exactly 0 in fp32.  Only active
nodes (m==1) contribute; for active pairs the mask term is 0.  The host
gathers active nodes per graph, the device computes the dense active x
active interaction, and the host scatters rows back (and applies the
1/n_active scale during the scatter).

Sharding: graphs sorted by active count, dealt round-robin to 8 cores in
4 slots; slot s padded to a global j-extent P_s (multiple of 4, for the
packed halving add) and an exact i-extent PI_s, so one SPMD program
serves all cores.  Padding columns get a -1e5 additive mask (sigmoid ->
0); padding rows are discarded on scatter.

Device design notes (from perfetto traces on HW):
- Hidden dim stays on partitions end-to-end; output is stored
  channel-major [128, 2*NTOT] and the host transposes, so there are no
  PE transposes and no per-slot PSUM evacuations.
- The DVE (vector engine) is the bottleneck.  fp32 tensor_tensor is
  1 elem/lane/cycle, but the 2x_1P packed bf16 mode works for the
  broadcast add if BOTH operands stream innermost step-1 4B-aligned:
  the ACT evacuation writes c' as duplicated bf16 pairs [c_i|c_i]
  (free: ACT is elem/cycle on its output) so in0 reads pairs step-1,
  y' is evacuated as plain bf16 rows.  Measured ~1.65 elem/cycle.
- The per-slot reduce runs as one bf16 2x halving add (st_lo+st_hi)
  plus a tensor_reduce of the half-width tensor (tensor_reduce never
  packs; this shaves ~25% of reduce cycles).
- GPSIMD compute is a trap: a running gpsimd tensor_tensor blocks any
  concurrently issued 2-port DVE op (shared POOL SBUF port) until it
  completes, and first use costs a ~3us ext-isa IRAM load.  GPSIMD only
  issues DMAs here.
- ACT does the 4 bias-adding PSUM evacuations and 4 per-slot fused-ob sigmoids
  ((352+FD)/1.2GHz each); both activation-table loads hide behind the
  input DMAs / matmuls.
- Input DMAs are split across the three DMA-capable queues (sync /
  scalar-ACT / gpsimd) with the first projections' operands first; DMA
  completion lags ~18ns per partition-row after issue, so weight chunks
  are [128, 256]-sized and ordered to unblock matmuls incrementally.
- bc/by are per-output-channel = per-partition here, so they ride the
  ACT evacuations as per-partition bias adds (no bias matmuls); only
  the pad-mask term keeps a rank-1 matmul (ones_H memset on device x
  madd row), so each projection's PSUM completes right after its two
  weight matmuls.
"""

import numpy as np

import concourse.bass as bass
import concourse.mybir as mybir
import concourse.tile as tile
from concourse import bacc
from concourse.bass_utils import run_bass_kernel_spmd

F32 = mybir.dt.float32
BF16 = mybir.dt.bfloat16
N_CORES = 8
PAD_NEG = -1.0e5  # additive mask for padding columns; sigmoid(-1e5) == 0

# test.py reads this for profiling info after a traced run
_last_results = None
_program_cache = {}

# slot index -> engine for the pairwise add ("v" = vector, "g" = gpsimd)
ADD_ENGINES = ("v", "v", "g", "g")


def _ap(view, free_dims):
    """AP anchored at `view`'s base with custom free dims (stride, num)."""
    return bass.AP(
        tensor=view.tensor,
        offset=view.offset,
        ap=[list(view.ap[0])] + [[int(s), int(n)] for s, n in free_dims],
    )


def _build_program(P_list, PI_list, H):
    """P_list: per-slot j-extent (mult of 4); PI_list: per-slot i-extent."""
    NTOT = sum(P_list)
    assert H == 256
    nc = bacc.Bacc(None, target_bir_lowering=False)

    zT = nc.dram_tensor("zT", [128, 2 * NTOT], BF16, kind="ExternalInput")
    # weight chunk layout: [128, (kb0ob0 | kb1ob0 | kb0ob1 | kb1ob1) * 128]
    wcT = nc.dram_tensor("wcT", [128, 2 * H], BF16, kind="ExternalInput")
    wyT = nc.dram_tensor("wyT", [128, 2 * H], BF16, kind="ExternalInput")
    biasv = nc.dram_tensor("biasv", [128, 4], F32, kind="ExternalInput")
    emr = nc.dram_tensor("emr", [1, NTOT], BF16, kind="ExternalInput")
    out = nc.dram_tensor("out", [128, 2 * NTOT], F32, kind="ExternalOutput")

    AT = mybir.ActivationFunctionType
    OP = mybir.AluOpType

    with tile.TileContext(nc) as tc:
        with (
            tc.tile_pool(name="singles", bufs=1) as singles,
            tc.tile_pool(name="ptp", bufs=2) as ptp,
            tc.tile_pool(name="stp", bufs=2) as stp,
            tc.tile_pool(name="hvp", bufs=2) as hvp,
            tc.tile_pool(name="oup", bufs=2) as oup,
            tc.tile_pool(name="psum", bufs=1, space="PSUM") as psum,
        ):
            em_sb = singles.tile([1, NTOT], BF16, tag="em", name="em_sb")
            z_sb = singles.tile([128, 2 * NTOT], BF16, tag="z", name="z_sb")
            w_sb = {
                "c": singles.tile([128, 2 * H], BF16, tag="wc", name="wc"),
                "y": singles.tile([128, 2 * H], BF16, tag="wy", name="wy"),
            }
            bias_sb = singles.tile([128, 4], F32, tag="biasv", name="bias_sb")
            ones_sb = singles.tile([1, H], BF16, tag="ones", name="ones_sb")
            nc.gpsimd.memset(ones_sb[:], 1.0)
            # operands for the first projections land first; three queues in
            # parallel, ob0 weight halves before ob1
            nc.sync.dma_start(out=z_sb[:, 0:NTOT], in_=zT[:, 0:NTOT])
            nc.gpsimd.dma_start(out=z_sb[:, NTOT:2 * NTOT], in_=zT[:, NTOT:2 * NTOT])
            nc.scalar.dma_start(out=bias_sb[:], in_=biasv[:, :])
            nc.scalar.dma_start(out=em_sb[:], in_=emr[:])
            nc.sync.dma_start(out=w_sb["y"][:, 0:256], in_=wyT[:, 0:256])
            nc.gpsimd.dma_start(out=w_sb["c"][:, 0:256], in_=wcT[:, 0:256])
            nc.sync.dma_start(out=w_sb["c"][:, 256:512], in_=wcT[:, 256:512])
            nc.gpsimd.dma_start(out=w_sb["y"][:, 256:512], in_=wyT[:, 256:512])

            # ---- projections -> PSUM; ACT evacuates y' as bf16 and c' as
            # duplicated bf16 pairs [c|c] (enables the 2x packed DVE add)
            c2 = singles.tile([128, 4 * NTOT], BF16, tag="c2", name="c2")
            yb = singles.tile([128, 2 * NTOT], BF16, tag="yb", name="yb")
            ps_t = {}
            for wname, ob in (("y", 0), ("c", 0), ("y", 1), ("c", 1)):
                ps = psum.tile(
                    [128, NTOT], F32, tag=f"ps{wname}{ob}", name=f"ps{wname}{ob}"
                )
                for kb in range(2):
                    o0 = (2 * ob + kb) * 128
                    nc.tensor.matmul(
                        ps[:],
                        lhsT=w_sb[wname][:, o0:o0 + 128],
                        rhs=z_sb[:, kb * NTOT:(kb + 1) * NTOT],
                        start=(kb == 0),
                        stop=(kb == 1 and wname == "c"),
                    )
                if wname == "y":
                    # pad-mask rank-1 term: ones_H (x) madd
                    nc.tensor.matmul(
                        ps[:], lhsT=ones_sb[:, ob * 128:(ob + 1) * 128],
                        rhs=em_sb[:], start=False, stop=True,
                    )
                ps_t[wname, ob] = ps
                # evacuation adds the per-channel (= per-partition) bias
                if wname == "y":
                    nc.scalar.activation(
                        out=yb[:, ob * NTOT:(ob + 1) * NTOT], in_=ps[:],
                        func=AT.Identity, bias=bias_sb[:, 2 + ob:3 + ob],
                    )
                else:
                    nc.scalar.activation(
                        out=_ap(c2[:, 2 * ob * NTOT:2 * ob * NTOT + 2],
                                [(2, NTOT), (1, 2)]),
                        in_=_ap(ps[:, 0:NTOT], [(1, NTOT), (0, 2)]),
                        func=AT.Identity, bias=bias_sb[:, ob:ob + 1],
                    )

            # ---- pass 1: all packed per-ob adds (slot-major)
            offs = [0]
            for P in P_list[:-1]:
                offs.append(offs[-1] + P)
            pts = []
            for si, (P, PI) in enumerate(zip(P_list, PI_list)):
                col = offs[si]
                pt = ptp.tile(
                    [128, 2, PI, P], BF16, tag=f"pair{si}", name=f"pair{si}"
                )
                for ob in range(2):
                    cb = 2 * ob * NTOT + 2 * col
                    in0 = _ap(c2[:, cb:cb + 2], [(2, PI), (0, P // 2), (1, 2)])
                    in1 = _ap(yb[:, ob * NTOT + col:ob * NTOT + col + P],
                              [(0, PI), (1, P)])
                    nc.vector.tensor_tensor(
                        out=pt[:, ob:ob + 1], in0=in0, in1=in1, op=OP.add
                    )
                pts.append(pt)

            # ---- pass 2: per-slot sigmoid -> halve -> reduce -> store
            for si, (P, PI) in enumerate(zip(P_list, PI_list)):
                col = offs[si]
                pt = pts[si]
                st = stp.tile([128, 2, PI, P], BF16, tag="sig", name="sig_t")
                nc.scalar.activation(out=st[:], in_=pt[:], func=AT.Sigmoid)
                hw = P // 2
                hv = hvp.tile([128, 2, PI, hw], BF16, tag="hv", name="hv_t")
                nc.vector.tensor_tensor(
                    out=hv[:], in0=st[:, :, :, 0:hw], in1=st[:, :, :, hw:P],
                    op=OP.add,
                )
                red = oup.tile([128, 2, PI], F32, tag="red", name="red_t")
                nc.vector.reduce_sum(out=red[:], in_=hv[:], axis=mybir.AxisListType.X)
                nc.sync.dma_start(
                    out=_ap(out[0:128, col:col + PI], [(NTOT, 2), (1, PI)]),
                    in_=red[:],
                )

    nc.finalize()
    return nc


def kernel(num_graphs, nv, z, mask, Wc, bc, Wy, by):
    global _last_results
    G = int(num_graphs)
    NV = int(nv)
    z = np.ascontiguousarray(np.asarray(z, dtype=np.float32))
    mask = np.asarray(mask, dtype=np.float32).reshape(G, NV)
    Wc = np.asarray(Wc, dtype=np.float32)
    bc = np.asarray(bc, dtype=np.float32)
    Wy = np.asarray(Wy, dtype=np.float32)
    by = np.asarray(by, dtype=np.float32)
    H = z.shape[-1]
    zg = z.reshape(G, NV, H)

    out_full = np.zeros((G * NV, H), dtype=np.float32)

    # ---- host: active-node compaction & slot assignment ----
    act_idx = [np.nonzero(mask[g] > 0.5)[0] for g in range(G)]
    n_act = np.array([len(a) for a in act_idx])
    for g in range(G):
        if n_act[g] == 0:  # reference: 0/0 -> NaN for the whole graph
            out_full[g * NV:(g + 1) * NV, :] = np.nan

    order = np.argsort(-n_act, kind="stable")  # graphs by count, descending
    n_slots = (G + N_CORES - 1) // N_CORES
    assign = [[None] * n_slots for _ in range(N_CORES)]
    P_list = []
    for s in range(n_slots):
        ranks = order[s * N_CORES:(s + 1) * N_CORES]
        for c, g in enumerate(ranks):
            assign[c][s] = int(g)
        mx = max((int(n_act[g]) for g in ranks), default=0)
        P_list.append(max(4, (mx + 3) // 4 * 4))  # j-extent: multiple of 4
    PI_list = [max(1, max((int(n_act[g]) for g in order[s * N_CORES:(s + 1) * N_CORES]), default=1)) for s in range(n_slots)]
    offs = np.cumsum([0] + P_list[:-1]).tolist()
    NTOT = sum(P_list)

    # ---- host: per-core input staging ----
    import ml_dtypes
    def _interleave(wt):  # [256, F] -> [128, 2*F] with kb blocks side by side
        f = wt.shape[1]
        w2 = np.empty((128, 2 * f), dtype=ml_dtypes.bfloat16)
        w2[:, :f] = wt[:128]
        w2[:, f:] = wt[128:]
        return np.ascontiguousarray(w2)

    def _wchunks(wt):  # [256, 256] -> [128, 512] chunks (kb,ob)-major for ob0 first
        w2 = np.empty((128, 512), dtype=ml_dtypes.bfloat16)
        for ob in range(2):
            for kb in range(2):
                w2[:, (2 * ob + kb) * 128:(2 * ob + kb + 1) * 128] = (
                    wt[kb * 128:(kb + 1) * 128, ob * 128:(ob + 1) * 128]
                )
        return np.ascontiguousarray(w2)

    wcT = _wchunks(Wc.T.astype(ml_dtypes.bfloat16))  # [h_in, o] chunks
    wyT = _wchunks(Wy.T.astype(ml_dtypes.bfloat16))
    biasv = np.ascontiguousarray(
        np.stack([bc[:128], bc[128:], by[:128], by[128:]], axis=1)
        .astype(np.float32)
    )

    in_maps = []
    for c in range(N_CORES):
        zT_act = np.zeros((H, NTOT), dtype=ml_dtypes.bfloat16)
        madd = np.full((1, NTOT), PAD_NEG, dtype=np.float32)
        for s in range(n_slots):
            g = assign[c][s]
            if g is None:
                continue
            n = int(n_act[g])
            if n == 0:
                continue
            o = int(offs[s])
            zT_act[:, o:o + n] = zg[g][act_idx[g]].T.astype(ml_dtypes.bfloat16)
            madd[0, o:o + n] = 0.0
        in_maps.append(
            {
                "zT": _interleave(zT_act),
                "wcT": wcT,
                "wyT": wyT,
                "biasv": biasv,
                "emr": np.ascontiguousarray(madd.astype(ml_dtypes.bfloat16)),
            }
        )

    # ---- build + run ----
    key = (tuple(P_list), tuple(PI_list), H)
    nc = _program_cache.get(key)
    if nc is None:
        nc = _build_program(P_list, PI_list, H)
        _program_cache[key] = nc
    res = run_bass_kernel_spmd(nc, in_maps, list(range(N_CORES)))
    _last_results = res

    # ---- host: scatter back (device output is [h1, (ob, col)]-major) ----
    for c in range(N_CORES):
        oc = res.results[c]["out"].reshape(128, 2, NTOT)  # [h1, ob, col]
        for s in range(n_slots):
            g = assign[c][s]
            if g is None:
                continue
            n = int(n_act[g])
            if n == 0:
                continue
            o = int(offs[s])
            blk = oc[:, :, o:o + n]  # [128, 2, n] (unscaled sums)
            out_full[g * NV + act_idx[g], :] = (
                blk.transpose(2, 1, 0).reshape(n, H)
                * (np.float32(1.0) / np.float32(n))
            )
    return out_full
